# revision 11
# baseline (speedup 1.0000x reference)
# CrystalAttention Trainium2 kernel.
#
# Full inputs -> shard batch dim over 8 NeuronCores -> bass/Tile kernel ->
# gather. Per core: x_sh [2048, 512].
#
#   dist2[n,m] = |x[m]|^2 + |pos[n]|^2 - 2 x[m].pos[n]
#   attn = softmax_n(scales[n] / (sqrt(dist2) + 0.1))
#   y = (attn @ values) @ W_out^T + b_out
#
# Layout: scores kept transposed [n on partitions, m free] so the softmax
# numerator matrix e feeds mm2 (lhsT=values, rhs=e) with no runtime
# transposes of the big [M,N] tensor. Softmax has no max-subtraction
# (scores = scales/(dist+0.1) ~ 0.22, exp is tiny and safe), and the
# normalization by the denominator is deferred to after out_proj (it is a
# per-row scalar).
#
# Matmuls run in bf16 (PE full rate). The distance's large constant is kept
# out of bf16: the augmentation row carries x_sq-512 and the fp32 Sqrt bias
# carries p_sq+512, so quantization error stays ~0.1% of the small residual
# rather than of ~516.
import numpy as np

B, T, D, N = 8, 2048, 512, 1024
NCORES = 8
P = 128
M = (B * T) // NCORES      # 2048 rows per core
MT = 512                   # m tile (matmul moving free dim)
N_MT = M // MT             # 4
DC = D // P                # 4 contraction chunks of 128
NCH = N // P               # 8 neuron chunks of 128
MS = MT // P               # 4 m-subtiles per m tile
XSQ_C = 512.0              # E[|x|^2] offset kept in fp32 bias

_CACHE = {}


def _build_bass(reps=1):
    import concourse.bacc as bacc
    import concourse.tile as tile
    import concourse.mybir as mybir
    from concourse.masks import make_identity

    fp32 = mybir.dt.float32
    bf16 = mybir.dt.bfloat16
    AF = mybir.ActivationFunctionType
    OP = mybir.AluOpType

    nc = bacc.Bacc(None, target_bir_lowering=False)

    x_d = nc.dram_tensor("x", [M, D], bf16, kind="ExternalInput")
    posn_d = nc.dram_tensor("posn", [D, N], bf16, kind="ExternalInput")   # -2*pos^T
    psq_d = nc.dram_tensor("psq", [P, NCH], fp32, kind="ExternalInput")   # |pos|^2+512
    invs_d = nc.dram_tensor("invs", [P, NCH], fp32, kind="ExternalInput")  # 1/scales
    offs_d = nc.dram_tensor("offs", [P, NCH], fp32, kind="ExternalInput")  # 0.1/scales
    val_d = nc.dram_tensor("vals", [N, D], bf16, kind="ExternalInput")
    wT_d = nc.dram_tensor("wT", [D, D], bf16, kind="ExternalInput")       # W_out^T
    bb_d = nc.dram_tensor("bb", [P, D], fp32, kind="ExternalInput")       # b_out bcast
    y_d = nc.dram_tensor("y", [M, D], fp32, kind="ExternalOutput")

    with tile.TileContext(nc) as tc:
        with (
            tc.tile_pool(name="const", bufs=1) as const,
            tc.tile_pool(name="big", bufs=1) as big,
            tc.tile_pool(name="sq", bufs=2) as sq_pool,
            tc.tile_pool(name="ut", bufs=2) as ut_pool,
            tc.tile_pool(name="small", bufs=3) as small,
            tc.tile_pool(name="yo", bufs=3) as y_pool,
            tc.tile_pool(name="ps_mm", bufs=6, space="PSUM") as ps_mm,
            tc.tile_pool(name="ps_row", bufs=1, space="PSUM") as ps_row,
            tc.tile_pool(name="ps_dt", bufs=1, space="PSUM") as ps_dt,
        ):
            # ---- constants / weights ----
            ident = const.tile([P, P], fp32)
            make_identity(nc, ident)
            ones_row = const.tile([1, P], bf16)    # aug stationary [k=1, n=128]
            nc.vector.memset(ones_row, 1.0)
            ones_col = const.tile([P, 1], bf16)    # reduce stationary [k=128, 1]
            nc.vector.memset(ones_col, 1.0)

            posn_sb = const.tile([P, DC, N], bf16)
            nc.sync.dma_start(posn_sb, posn_d.rearrange("(c p) n -> p c n", p=P))
            psq_sb = const.tile([P, NCH], fp32)
            nc.sync.dma_start(psq_sb, psq_d[:])
            invs_sb = const.tile([P, NCH], fp32)
            nc.sync.dma_start(invs_sb, invs_d[:])
            offs_sb = const.tile([P, NCH], fp32)
            nc.sync.dma_start(offs_sb, offs_d[:])
            val_sb = const.tile([P, NCH, D], bf16)
            nc.sync.dma_start(val_sb, val_d.rearrange("(c p) d -> p c d", p=P))
            wT_sb = const.tile([P, DC, D], bf16)
            nc.sync.dma_start(wT_sb, wT_d.rearrange("(c p) o -> p c o", p=P))
            bb_sb = const.tile([P, D], fp32)
            nc.sync.dma_start(bb_sb, bb_d[:])

            xT_sb = big.tile([P, DC, M], bf16)      # x transposed [d, m]
            score_sb = big.tile([P, NCH, M], fp32)  # dist -> score
            e_sb = big.tile([P, NCH, M], bf16)      # exp(score)
            xsq_sb = big.tile([1, M], bf16)         # |x[m]|^2 - 512 row

            import contextlib
            loop_cm = (
                tc.For_i(0, reps, 1, hint_engines=(mybir.EngineType.PE,))
                if reps > 1 else contextlib.nullcontext()
            )
            sqrt_instrs = []

            # ================= phase A: distances + scores =================
            stack = contextlib.ExitStack()
            stack.enter_context(loop_cm)
            for t in range(N_MT):
                mt = slice(t * MT, (t + 1) * MT)
                # transpose x [m,d] -> xT [d,m] via XBAR DMA (bf16)
                nc.sync.dma_start_transpose(
                    xT_sb[:, :, mt], x_d[t * MT:(t + 1) * MT, :]
                )

                # |x|^2 row: square then ones-reduce over partitions
                sqt = sq_pool.tile([P, DC, MT], bf16, tag="sq")
                nc.vector.tensor_tensor(
                    sqt, xT_sb[:, :, mt], xT_sb[:, :, mt], op=OP.mult
                )
                xq = ps_row.tile([1, MT], fp32, tag="row")
                for dc in range(DC):
                    nc.tensor.matmul(
                        xq, ones_col, sqt[:, dc, :],
                        start=(dc == 0), stop=(dc == DC - 1),
                    )
                # xsq residual row in bf16: x_sq - 512
                nc.vector.tensor_scalar(
                    xsq_sb[:, mt], xq, -XSQ_C, None, op0=OP.add
                )

                # mm1 per neuron chunk: d2 = xsq + psq - 2 x.pos, then score
                for c in range(NCH):
                    d2 = ps_mm.tile([P, MT], fp32, tag="mm")
                    for dc in range(DC):
                        nc.tensor.matmul(
                            d2,
                            posn_sb[:, dc, c * P:(c + 1) * P],
                            xT_sb[:, dc, mt],
                            start=(dc == 0), stop=False,
                        )
                    nc.tensor.matmul(
                        d2, ones_row, xsq_sb[:, mt], start=False, stop=True
                    )
                    sc = score_sb[:, c, mt]
                    si = nc.scalar.activation(
                        sc, d2, AF.Sqrt, bias=psq_sb[:, c:c + 1], scale=1.0
                    )
                    sqrt_instrs.append(si)
                    # v = (dist + 0.1)/scales  (per-partition scalars)
                    nc.gpsimd.tensor_scalar(
                        sc, sc, invs_sb[:, c:c + 1], offs_sb[:, c:c + 1],
                        op0=OP.mult, op1=OP.add,
                    )

                # score = 1/v, one batched op over all neuron chunks
                nc.vector.reciprocal_approx_fast(
                    score_sb[:, :, mt], score_sb[:, :, mt]
                )

            # ================= phase B: exp + attn@values + out_proj =======
            exp_instrs = []
            for t in range(N_MT):
                mt = slice(t * MT, (t + 1) * MT)
                ei = nc.scalar.activation(
                    e_sb[:, :, mt], score_sb[:, :, mt], AF.Exp
                )
                exp_instrs.append(ei)

                # uT[d, m] = sum_n values[n, d] e[n, m]
                ut = ut_pool.tile([P, DC, MT], bf16, tag="ut")
                for ds in range(DC):
                    u = ps_mm.tile([P, MT], fp32, tag="mm")
                    for c in range(NCH):
                        nc.tensor.matmul(
                            u,
                            val_sb[:, c, ds * P:(ds + 1) * P],
                            e_sb[:, c, mt],
                            start=(c == 0), stop=(c == NCH - 1),
                        )
                    nc.scalar.activation(ut[:, ds, :], u, AF.Copy)

                # denom[m] = sum_n e[n, m]
                den = ps_row.tile([1, MT], fp32, tag="row")
                for c in range(NCH):
                    nc.tensor.matmul(
                        den, ones_col, e_sb[:, c, mt],
                        start=(c == 0), stop=(c == NCH - 1),
                    )
                den_sb = small.tile([1, MT], fp32, tag="densb")
                nc.scalar.activation(den_sb, den, AF.Copy)
                # transpose denom row into [m partitions, 1] and invert
                dt_ps = ps_dt.tile([P, MS], fp32, tag="dt")
                for j in range(MS):
                    nc.tensor.transpose(
                        dt_ps[:, j:j + 1], den_sb[:, j * P:(j + 1) * P],
                        ident[0:1, 0:1],
                    )
                rden = small.tile([P, MS], fp32, tag="rden")
                nc.vector.reciprocal_approx_fast(rden, dt_ps)

                # y = (uT^T @ W^T) * (1/denom) + b
                for ms in range(MS):
                    yps = ps_mm.tile([P, D], fp32, tag="mm")
                    for dc in range(DC):
                        nc.tensor.matmul(
                            yps,
                            ut[:, dc, ms * P:(ms + 1) * P],
                            wT_sb[:, dc, :],
                            start=(dc == 0), stop=(dc == DC - 1),
                        )
                    yt = y_pool.tile([P, D], fp32, tag="y")
                    nc.vector.scalar_tensor_tensor(
                        yt, yps, rden[:, ms:ms + 1], bb_sb,
                        op0=OP.mult, op1=OP.add,
                    )
                    r0 = t * MT + ms * P
                    nc.sync.dma_start(y_d[r0:r0 + P, :], yt)

            # keep every Exp after the last Sqrt on the ACT stream so the
            # activation table set switches exactly once
            last_sqrt = sqrt_instrs[-1]
            for ei in exp_instrs:
                tile.add_dep_helper(
                    ei.ins, last_sqrt.ins, sync=False, reason="act table phase"
                )
            stack.close()

    nc.compile()
    return nc


def _get_bass(reps=1):
    key = ("nc", reps)
    if key not in _CACHE:
        _CACHE[key] = _build_bass(reps)
    return _CACHE[key]


def _host_prep(positions, scales, values, W_out, b_out):
    import ml_dtypes

    bf16 = ml_dtypes.bfloat16
    positions = positions.astype(np.float32)
    posn = np.ascontiguousarray((-2.0 * positions.T).astype(bf16))
    psq = np.ascontiguousarray(
        ((positions ** 2).sum(-1) + XSQ_C).astype(np.float32).reshape(NCH, P).T
    )
    invs = np.ascontiguousarray((1.0 / scales).astype(np.float32).reshape(NCH, P).T)
    offs = np.ascontiguousarray((0.1 / scales).astype(np.float32).reshape(NCH, P).T)
    vals = np.ascontiguousarray(values.astype(bf16))
    wT = np.ascontiguousarray(W_out.astype(np.float32).T.astype(bf16))
    bb = np.ascontiguousarray(np.tile(b_out.astype(np.float32)[None, :], (P, 1)))
    return posn, psq, invs, offs, vals, wT, bb


def kernel(x, positions, scales, values, W_out, b_out, _trace=False, _tmpdir=None,
           _reps=1):
    from concourse.bass_utils import run_bass_kernel_spmd

    import ml_dtypes
    x = np.asarray(x, dtype=np.float32).reshape(B * T, D).astype(ml_dtypes.bfloat16)
    posn, psq, invs, offs, vals, wT, bb = _host_prep(
        np.asarray(positions), np.asarray(scales), np.asarray(values),
        np.asarray(W_out), np.asarray(b_out),
    )

    shards = x.reshape(NCORES, M, D)
    in_maps = [
        {
            "x": np.ascontiguousarray(shards[i]),
            "posn": posn, "psq": psq, "invs": invs, "offs": offs,
            "vals": vals, "wT": wT, "bb": bb,
        }
        for i in range(NCORES)
    ]

    nc = _get_bass(_reps)
    res = run_bass_kernel_spmd(
        nc, in_maps, core_ids=list(range(NCORES)),
        trace=_trace, tmpdir=_tmpdir,
    )
    y = np.stack([res.results[i]["y"] for i in range(NCORES)], axis=0)
    if _trace:
        _CACHE["last_result"] = res
    return y.reshape(B, T, D).astype(np.float32)


# revision 12
# speedup vs baseline: 1.0683x; 1.0683x over previous
# CrystalAttention Trainium2 kernel.
#
# Full inputs -> shard batch dim over 8 NeuronCores -> bass/Tile kernel ->
# gather. Per core: x_sh [2048, 512].
#
#   dist2[n,m] = |x[m]|^2 + |pos[n]|^2 - 2 x[m].pos[n]
#   attn = softmax_n(scales[n] / (sqrt(dist2) + 0.1))
#   y = (attn @ values) @ W_out^T + b_out
#
# Layout: scores kept transposed [n on partitions, m free] so the softmax
# numerator matrix e feeds mm2 (lhsT=values, rhs=e) with no runtime
# transposes of the big [M,N] tensor. Softmax has no max-subtraction
# (scores = scales/(dist+0.1) ~ 0.22, exp is tiny and safe), and the
# normalization by the denominator is deferred to after out_proj (it is a
# per-row scalar).
#
# Matmuls run in bf16 (PE full rate). The distance's large constant is kept
# out of bf16: the augmentation row carries x_sq-512 and the fp32 Sqrt bias
# carries p_sq+512, so quantization error stays ~0.1% of the small residual
# rather than of ~516.
import numpy as np

B, T, D, N = 8, 2048, 512, 1024
NCORES = 8
P = 128
M = (B * T) // NCORES      # 2048 rows per core
MT = 512                   # m tile (matmul moving free dim)
N_MT = M // MT             # 4
DC = D // P                # 4 contraction chunks of 128
NCH = N // P               # 8 neuron chunks of 128
MS = MT // P               # 4 m-subtiles per m tile
XSQ_C = 512.0              # E[|x|^2] offset kept in fp32 bias

_CACHE = {}


def _build_bass(reps=1):
    import concourse.bacc as bacc
    import concourse.tile as tile
    import concourse.mybir as mybir
    from concourse.masks import make_identity

    fp32 = mybir.dt.float32
    bf16 = mybir.dt.bfloat16
    AF = mybir.ActivationFunctionType
    OP = mybir.AluOpType

    nc = bacc.Bacc(None, target_bir_lowering=False)

    x_d = nc.dram_tensor("x", [M, D], bf16, kind="ExternalInput")
    posn_d = nc.dram_tensor("posn", [D, N], bf16, kind="ExternalInput")   # -2*pos^T
    psq_d = nc.dram_tensor("psq", [P, NCH], fp32, kind="ExternalInput")   # |pos|^2+512
    invs_d = nc.dram_tensor("invs", [P, NCH], fp32, kind="ExternalInput")  # 1/scales
    offs_d = nc.dram_tensor("offs", [P, NCH], fp32, kind="ExternalInput")  # 0.1/scales
    val_d = nc.dram_tensor("vals", [N, D], bf16, kind="ExternalInput")
    wT_d = nc.dram_tensor("wT", [D, D], bf16, kind="ExternalInput")       # W_out^T
    bb_d = nc.dram_tensor("bb", [P, D], fp32, kind="ExternalInput")       # b_out bcast
    y_d = nc.dram_tensor("y", [M, D], fp32, kind="ExternalOutput")

    with tile.TileContext(nc) as tc:
        with (
            tc.tile_pool(name="const", bufs=1) as const,
            tc.tile_pool(name="big", bufs=1) as big,
            tc.tile_pool(name="sq", bufs=2) as sq_pool,
            tc.tile_pool(name="ut", bufs=2) as ut_pool,
            tc.tile_pool(name="small", bufs=3) as small,
            tc.tile_pool(name="yo", bufs=3) as y_pool,
            tc.tile_pool(name="ps_mm", bufs=6, space="PSUM") as ps_mm,
            tc.tile_pool(name="ps_row", bufs=1, space="PSUM") as ps_row,
            tc.tile_pool(name="ps_dt", bufs=1, space="PSUM") as ps_dt,
        ):
            # ---- constants / weights ----
            ident = const.tile([P, P], fp32)
            make_identity(nc, ident)
            ones_row = const.tile([1, P], bf16)    # aug stationary [k=1, n=128]
            nc.vector.memset(ones_row, 1.0)
            ones_col = const.tile([P, 1], bf16)    # reduce stationary [k=128, 1]
            nc.vector.memset(ones_col, 1.0)

            posn_sb = const.tile([P, DC, N], bf16)
            nc.sync.dma_start(posn_sb, posn_d.rearrange("(c p) n -> p c n", p=P))
            psq_sb = const.tile([P, NCH], fp32)
            nc.sync.dma_start(psq_sb, psq_d[:])
            invs_sb = const.tile([P, NCH], fp32)
            nc.sync.dma_start(invs_sb, invs_d[:])
            offs_sb = const.tile([P, NCH], fp32)
            nc.sync.dma_start(offs_sb, offs_d[:])
            val_sb = const.tile([P, NCH, D], bf16)
            nc.sync.dma_start(val_sb, val_d.rearrange("(c p) d -> p c d", p=P))
            wT_sb = const.tile([P, DC, D], bf16)
            nc.sync.dma_start(wT_sb, wT_d.rearrange("(c p) o -> p c o", p=P))
            bb_sb = const.tile([P, D], fp32)
            nc.sync.dma_start(bb_sb, bb_d[:])

            xT_sb = big.tile([P, DC, M], bf16)      # x transposed [d, m]
            score_sb = big.tile([P, NCH, M], fp32)  # dist -> score
            e_sb = big.tile([P, NCH, M], bf16)      # exp(score)
            xsq_sb = big.tile([1, M], bf16)         # |x[m]|^2 - 512 row

            import contextlib
            loop_cm = (
                tc.For_i(0, reps, 1, hint_engines=(mybir.EngineType.PE,))
                if reps > 1 else contextlib.nullcontext()
            )
            sqrt_instrs = []

            # ================= phase A: distances + scores =================
            stack = contextlib.ExitStack()
            stack.enter_context(loop_cm)
            for t in range(N_MT):
                mt = slice(t * MT, (t + 1) * MT)
                # transpose x [m,d] -> xT [d,m] via XBAR DMA (bf16)
                nc.sync.dma_start_transpose(
                    xT_sb[:, :, mt], x_d[t * MT:(t + 1) * MT, :]
                )

                # |x|^2 row: square then ones-reduce over partitions
                sqt = sq_pool.tile([P, DC, MT], bf16, tag="sq")
                nc.vector.tensor_tensor(
                    sqt, xT_sb[:, :, mt], xT_sb[:, :, mt], op=OP.mult
                )
                xq = ps_row.tile([1, MT], fp32, tag="row")
                for dc in range(DC):
                    nc.tensor.matmul(
                        xq, ones_col, sqt[:, dc, :],
                        start=(dc == 0), stop=(dc == DC - 1),
                    )
                # xsq residual row in bf16: x_sq - 512
                nc.vector.tensor_scalar(
                    xsq_sb[:, mt], xq, -XSQ_C, None, op0=OP.add
                )

                # mm1 per neuron chunk: d2 = xsq + psq - 2 x.pos, then score
                for c in range(NCH):
                    d2 = ps_mm.tile([P, MT], fp32, tag="mm")
                    for dc in range(DC):
                        nc.tensor.matmul(
                            d2,
                            posn_sb[:, dc, c * P:(c + 1) * P],
                            xT_sb[:, dc, mt],
                            start=(dc == 0), stop=False,
                        )
                    nc.tensor.matmul(
                        d2, ones_row, xsq_sb[:, mt], start=False, stop=True
                    )
                    sc = score_sb[:, c, mt]
                    si = nc.scalar.activation(
                        sc, d2, AF.Sqrt, bias=psq_sb[:, c:c + 1], scale=1.0
                    )
                    sqrt_instrs.append(si)
                    # v = (dist + 0.1)/scales  (per-partition scalars)
                    nc.gpsimd.tensor_scalar(
                        sc, sc, invs_sb[:, c:c + 1], offs_sb[:, c:c + 1],
                        op0=OP.mult, op1=OP.add,
                    )
                    nc.vector.reciprocal_approx_fast(sc, sc)

            # ================= phase B: exp + attn@values + out_proj =======
            exp_instrs = []
            for t in range(N_MT):
                mt = slice(t * MT, (t + 1) * MT)
                ei = nc.scalar.activation(
                    e_sb[:, :, mt], score_sb[:, :, mt], AF.Exp
                )
                exp_instrs.append(ei)

                # uT[d, m] = sum_n values[n, d] e[n, m]
                ut = ut_pool.tile([P, DC, MT], bf16, tag="ut")
                for ds in range(DC):
                    u = ps_mm.tile([P, MT], fp32, tag="mm")
                    for c in range(NCH):
                        nc.tensor.matmul(
                            u,
                            val_sb[:, c, ds * P:(ds + 1) * P],
                            e_sb[:, c, mt],
                            start=(c == 0), stop=(c == NCH - 1),
                        )
                    nc.scalar.activation(ut[:, ds, :], u, AF.Copy)

                # denom[m] = sum_n e[n, m]
                den = ps_row.tile([1, MT], fp32, tag="row")
                for c in range(NCH):
                    nc.tensor.matmul(
                        den, ones_col, e_sb[:, c, mt],
                        start=(c == 0), stop=(c == NCH - 1),
                    )
                den_sb = small.tile([1, MT], fp32, tag="densb")
                nc.scalar.activation(den_sb, den, AF.Copy)
                # transpose denom row into [m partitions, 1] and invert
                dt_ps = ps_dt.tile([P, MS], fp32, tag="dt")
                for j in range(MS):
                    nc.tensor.transpose(
                        dt_ps[:, j:j + 1], den_sb[:, j * P:(j + 1) * P],
                        ident[0:1, 0:1],
                    )
                rden = small.tile([P, MS], fp32, tag="rden")
                nc.vector.reciprocal_approx_fast(rden, dt_ps)

                # y = (uT^T @ W^T) * (1/denom) + b
                for ms in range(MS):
                    yps = ps_mm.tile([P, D], fp32, tag="mm")
                    for dc in range(DC):
                        nc.tensor.matmul(
                            yps,
                            ut[:, dc, ms * P:(ms + 1) * P],
                            wT_sb[:, dc, :],
                            start=(dc == 0), stop=(dc == DC - 1),
                        )
                    yt = y_pool.tile([P, D], fp32, tag="y")
                    nc.vector.scalar_tensor_tensor(
                        yt, yps, rden[:, ms:ms + 1], bb_sb,
                        op0=OP.mult, op1=OP.add,
                    )
                    r0 = t * MT + ms * P
                    nc.sync.dma_start(y_d[r0:r0 + P, :], yt)

            # keep every Exp after the last Sqrt on the ACT stream so the
            # activation table set switches exactly once
            last_sqrt = sqrt_instrs[-1]
            for ei in exp_instrs:
                tile.add_dep_helper(
                    ei.ins, last_sqrt.ins, sync=False, reason="act table phase"
                )
            stack.close()

    nc.compile()
    return nc


def _get_bass(reps=1):
    key = ("nc", reps)
    if key not in _CACHE:
        _CACHE[key] = _build_bass(reps)
    return _CACHE[key]


def _host_prep(positions, scales, values, W_out, b_out):
    import ml_dtypes

    bf16 = ml_dtypes.bfloat16
    positions = positions.astype(np.float32)
    posn = np.ascontiguousarray((-2.0 * positions.T).astype(bf16))
    psq = np.ascontiguousarray(
        ((positions ** 2).sum(-1) + XSQ_C).astype(np.float32).reshape(NCH, P).T
    )
    invs = np.ascontiguousarray((1.0 / scales).astype(np.float32).reshape(NCH, P).T)
    offs = np.ascontiguousarray((0.1 / scales).astype(np.float32).reshape(NCH, P).T)
    vals = np.ascontiguousarray(values.astype(bf16))
    wT = np.ascontiguousarray(W_out.astype(np.float32).T.astype(bf16))
    bb = np.ascontiguousarray(np.tile(b_out.astype(np.float32)[None, :], (P, 1)))
    return posn, psq, invs, offs, vals, wT, bb


def kernel(x, positions, scales, values, W_out, b_out, _trace=False, _tmpdir=None,
           _reps=1):
    from concourse.bass_utils import run_bass_kernel_spmd

    import ml_dtypes
    x = np.asarray(x, dtype=np.float32).reshape(B * T, D).astype(ml_dtypes.bfloat16)
    posn, psq, invs, offs, vals, wT, bb = _host_prep(
        np.asarray(positions), np.asarray(scales), np.asarray(values),
        np.asarray(W_out), np.asarray(b_out),
    )

    shards = x.reshape(NCORES, M, D)
    in_maps = [
        {
            "x": np.ascontiguousarray(shards[i]),
            "posn": posn, "psq": psq, "invs": invs, "offs": offs,
            "vals": vals, "wT": wT, "bb": bb,
        }
        for i in range(NCORES)
    ]

    nc = _get_bass(_reps)
    res = run_bass_kernel_spmd(
        nc, in_maps, core_ids=list(range(NCORES)),
        trace=_trace, tmpdir=_tmpdir,
    )
    y = np.stack([res.results[i]["y"] for i in range(NCORES)], axis=0)
    if _trace:
        _CACHE["last_result"] = res
    return y.reshape(B, T, D).astype(np.float32)


# revision 13
# speedup vs baseline: 1.7587x; 1.6463x over previous
# CrystalAttention Trainium2 kernel.
#
# Full inputs -> shard batch dim over 8 NeuronCores -> bass/Tile kernel ->
# gather. Per core: x_sh [2048, 512].
#
#   dist2[n,m] = |x[m]|^2 + |pos[n]|^2 - 2 x[m].pos[n]
#   attn = softmax_n(scales[n] / (sqrt(dist2) + 0.1))
#   y = (attn @ values) @ W_out^T + b_out
#
# Layout: scores kept transposed [n on partitions, m free] so the softmax
# numerator matrix e feeds mm2 (lhsT=values, rhs=e) with no runtime
# transposes of the big [M,N] tensor. Softmax has no max-subtraction
# (scores = scales/(dist+0.1) ~ 0.22, exp is tiny and safe), and the
# normalization by the denominator is deferred to after out_proj (it is a
# per-row scalar).
#
# Matmuls run in bf16 (PE full rate). The distance's large constant is kept
# out of bf16: the augmentation row carries x_sq-512 and the fp32 Sqrt bias
# carries p_sq+512, so quantization error stays ~0.1% of the small residual
# rather than of ~516.
import numpy as np

B, T, D, N = 8, 2048, 512, 1024
NCORES = 8
P = 128
M = (B * T) // NCORES      # 2048 rows per core
MT = 512                   # m tile (matmul moving free dim)
N_MT = M // MT             # 4
DC = D // P                # 4 contraction chunks of 128
NCH = N // P               # 8 neuron chunks of 128
MS = MT // P               # 4 m-subtiles per m tile
XSQ_C = 512.0              # E[|x|^2] offset kept in fp32 bias

_CACHE = {}


def _build_bass(reps=1):
    import concourse.bacc as bacc
    import concourse.tile as tile
    import concourse.mybir as mybir
    from concourse.masks import make_identity

    fp32 = mybir.dt.float32
    bf16 = mybir.dt.bfloat16
    AF = mybir.ActivationFunctionType
    OP = mybir.AluOpType

    nc = bacc.Bacc(None, target_bir_lowering=False)

    x_d = nc.dram_tensor("x", [M, D], bf16, kind="ExternalInput")
    posn_d = nc.dram_tensor("posn", [D, N], bf16, kind="ExternalInput")   # -2*pos^T
    psq_d = nc.dram_tensor("psq", [P, NCH], fp32, kind="ExternalInput")   # |pos|^2+512
    invs_d = nc.dram_tensor("invs", [P, NCH], fp32, kind="ExternalInput")  # 1/scales
    offs_d = nc.dram_tensor("offs", [P, NCH], fp32, kind="ExternalInput")  # 0.1/scales
    val_d = nc.dram_tensor("vals", [N, D], bf16, kind="ExternalInput")
    wT_d = nc.dram_tensor("wT", [D, D], bf16, kind="ExternalInput")       # W_out^T
    bb_d = nc.dram_tensor("bb", [P, D], fp32, kind="ExternalInput")       # b_out bcast
    y_d = nc.dram_tensor("y", [M, D], fp32, kind="ExternalOutput")

    with tile.TileContext(nc) as tc:
        with (
            tc.tile_pool(name="const", bufs=1) as const,
            tc.tile_pool(name="big", bufs=1) as big,
            tc.tile_pool(name="sq", bufs=2) as sq_pool,
            tc.tile_pool(name="ut", bufs=2) as ut_pool,
            tc.tile_pool(name="small", bufs=3) as small,
            tc.tile_pool(name="yo", bufs=3) as y_pool,
            tc.tile_pool(name="ps_mm", bufs=6, space="PSUM") as ps_mm,
            tc.tile_pool(name="ps_row", bufs=1, space="PSUM") as ps_row,
            tc.tile_pool(name="ps_dt", bufs=1, space="PSUM") as ps_dt,
        ):
            # ---- constants / weights ----
            ident = const.tile([P, P], fp32)
            make_identity(nc, ident)
            ones_row = const.tile([1, P], bf16)    # aug stationary [k=1, n=128]
            nc.vector.memset(ones_row, 1.0)
            ones_col = const.tile([P, 1], bf16)    # reduce stationary [k=128, 1]
            nc.vector.memset(ones_col, 1.0)

            posn_sb = const.tile([P, DC, N], bf16)
            nc.sync.dma_start(posn_sb, posn_d.rearrange("(c p) n -> p c n", p=P))
            psq_sb = const.tile([P, NCH], fp32)
            nc.sync.dma_start(psq_sb, psq_d[:])
            invs_sb = const.tile([P, NCH], fp32)
            nc.sync.dma_start(invs_sb, invs_d[:])
            offs_sb = const.tile([P, NCH], fp32)
            nc.sync.dma_start(offs_sb, offs_d[:])
            val_sb = const.tile([P, NCH, D], bf16)
            nc.sync.dma_start(val_sb, val_d.rearrange("(c p) d -> p c d", p=P))
            wT_sb = const.tile([P, DC, D], bf16)
            nc.sync.dma_start(wT_sb, wT_d.rearrange("(c p) o -> p c o", p=P))
            bb_sb = const.tile([P, D], fp32)
            nc.sync.dma_start(bb_sb, bb_d[:])

            xT_sb = big.tile([P, DC, M], bf16)      # x transposed [d, m]
            score_sb = big.tile([P, NCH, M], fp32)  # dist -> score
            e_sb = big.tile([P, NCH, M], bf16)      # exp(score)
            xsq_sb = big.tile([1, M], bf16)         # |x[m]|^2 - 512 row

            import contextlib
            loop_cm = (
                tc.For_i(0, reps, 1, hint_engines=(mybir.EngineType.PE,))
                if reps > 1 else contextlib.nullcontext()
            )
            sqrt_instrs = []

            # ================= phase A: distances + scores =================
            stack = contextlib.ExitStack()
            stack.enter_context(loop_cm)
            for t in range(N_MT):
                mt = slice(t * MT, (t + 1) * MT)
                # transpose x [m,d] -> xT [d,m] via XBAR DMA (bf16)
                nc.sync.dma_start_transpose(
                    xT_sb[:, :, mt], x_d[t * MT:(t + 1) * MT, :]
                )

                # |x|^2 row: square then ones-reduce over partitions
                sqt = sq_pool.tile([P, DC, MT], bf16, tag="sq")
                nc.scalar.activation(sqt, xT_sb[:, :, mt], AF.Square)
                xq = ps_row.tile([1, MT], fp32, tag="row")
                for dc in range(DC):
                    nc.tensor.matmul(
                        xq, ones_col, sqt[:, dc, :],
                        start=(dc == 0), stop=(dc == DC - 1),
                    )
                # xsq residual row in bf16: x_sq - 512
                nc.vector.tensor_scalar(
                    xsq_sb[:, mt], xq, -XSQ_C, None, op0=OP.add
                )

                # mm1 per neuron chunk: d2 = xsq + psq - 2 x.pos, then score
                for c in range(NCH):
                    d2 = ps_mm.tile([P, MT], fp32, tag="mm")
                    for dc in range(DC):
                        nc.tensor.matmul(
                            d2,
                            posn_sb[:, dc, c * P:(c + 1) * P],
                            xT_sb[:, dc, mt],
                            start=(dc == 0), stop=False,
                        )
                    nc.tensor.matmul(
                        d2, ones_row, xsq_sb[:, mt], start=False, stop=True
                    )
                    sc = score_sb[:, c, mt]
                    si = nc.scalar.activation(
                        sc, d2, AF.Sqrt, bias=psq_sb[:, c:c + 1], scale=1.0
                    )
                    sqrt_instrs.append(si)
                    # v = (dist + 0.1)/scales  (per-partition scalars)
                    nc.gpsimd.tensor_scalar(
                        sc, sc, invs_sb[:, c:c + 1], offs_sb[:, c:c + 1],
                        op0=OP.mult, op1=OP.add,
                    )
                    nc.vector.reciprocal_approx_fast(sc, sc)

            # ================= phase B: exp + attn@values + out_proj =======
            exp_instrs = []
            for t in range(N_MT):
                mt = slice(t * MT, (t + 1) * MT)
                ei = nc.scalar.activation(
                    e_sb[:, :, mt], score_sb[:, :, mt], AF.Exp
                )
                exp_instrs.append(ei)

                # uT[d, m] = sum_n values[n, d] e[n, m]
                ut = ut_pool.tile([P, DC, MT], bf16, tag="ut")
                for ds in range(DC):
                    u = ps_mm.tile([P, MT], fp32, tag="mm")
                    for c in range(NCH):
                        nc.tensor.matmul(
                            u,
                            val_sb[:, c, ds * P:(ds + 1) * P],
                            e_sb[:, c, mt],
                            start=(c == 0), stop=(c == NCH - 1),
                        )
                    nc.scalar.activation(ut[:, ds, :], u, AF.Copy)

                # denom[m] = sum_n e[n, m]
                den = ps_row.tile([1, MT], fp32, tag="row")
                for c in range(NCH):
                    nc.tensor.matmul(
                        den, ones_col, e_sb[:, c, mt],
                        start=(c == 0), stop=(c == NCH - 1),
                    )
                den_sb = small.tile([1, MT], fp32, tag="densb")
                nc.scalar.activation(den_sb, den, AF.Copy)
                # transpose denom row into [m partitions, 1] and invert
                dt_ps = ps_dt.tile([P, MS], fp32, tag="dt")
                for j in range(MS):
                    nc.tensor.transpose(
                        dt_ps[:, j:j + 1], den_sb[:, j * P:(j + 1) * P],
                        ident[0:1, 0:1],
                    )
                rden = small.tile([P, MS], fp32, tag="rden")
                nc.vector.reciprocal_approx_fast(rden, dt_ps)

                # y = (uT^T @ W^T) * (1/denom) + b
                for ms in range(MS):
                    yps = ps_mm.tile([P, D], fp32, tag="mm")
                    for dc in range(DC):
                        nc.tensor.matmul(
                            yps,
                            ut[:, dc, ms * P:(ms + 1) * P],
                            wT_sb[:, dc, :],
                            start=(dc == 0), stop=(dc == DC - 1),
                        )
                    yt = y_pool.tile([P, D], fp32, tag="y")
                    nc.vector.scalar_tensor_tensor(
                        yt, yps, rden[:, ms:ms + 1], bb_sb,
                        op0=OP.mult, op1=OP.add,
                    )
                    r0 = t * MT + ms * P
                    nc.sync.dma_start(y_d[r0:r0 + P, :], yt)

            # keep every Exp after the last Sqrt on the ACT stream so the
            # activation table set switches exactly once
            last_sqrt = sqrt_instrs[-1]
            for ei in exp_instrs:
                tile.add_dep_helper(
                    ei.ins, last_sqrt.ins, sync=False, reason="act table phase"
                )
            stack.close()

    nc.compile()
    return nc


def _get_bass(reps=1):
    key = ("nc", reps)
    if key not in _CACHE:
        _CACHE[key] = _build_bass(reps)
    return _CACHE[key]


def _host_prep(positions, scales, values, W_out, b_out):
    import ml_dtypes

    bf16 = ml_dtypes.bfloat16
    positions = positions.astype(np.float32)
    posn = np.ascontiguousarray((-2.0 * positions.T).astype(bf16))
    psq = np.ascontiguousarray(
        ((positions ** 2).sum(-1) + XSQ_C).astype(np.float32).reshape(NCH, P).T
    )
    invs = np.ascontiguousarray((1.0 / scales).astype(np.float32).reshape(NCH, P).T)
    offs = np.ascontiguousarray((0.1 / scales).astype(np.float32).reshape(NCH, P).T)
    vals = np.ascontiguousarray(values.astype(bf16))
    wT = np.ascontiguousarray(W_out.astype(np.float32).T.astype(bf16))
    bb = np.ascontiguousarray(np.tile(b_out.astype(np.float32)[None, :], (P, 1)))
    return posn, psq, invs, offs, vals, wT, bb


def kernel(x, positions, scales, values, W_out, b_out, _trace=False, _tmpdir=None,
           _reps=1):
    from concourse.bass_utils import run_bass_kernel_spmd

    import ml_dtypes
    x = np.asarray(x, dtype=np.float32).reshape(B * T, D).astype(ml_dtypes.bfloat16)
    posn, psq, invs, offs, vals, wT, bb = _host_prep(
        np.asarray(positions), np.asarray(scales), np.asarray(values),
        np.asarray(W_out), np.asarray(b_out),
    )

    shards = x.reshape(NCORES, M, D)
    in_maps = [
        {
            "x": np.ascontiguousarray(shards[i]),
            "posn": posn, "psq": psq, "invs": invs, "offs": offs,
            "vals": vals, "wT": wT, "bb": bb,
        }
        for i in range(NCORES)
    ]

    nc = _get_bass(_reps)
    res = run_bass_kernel_spmd(
        nc, in_maps, core_ids=list(range(NCORES)),
        trace=_trace, tmpdir=_tmpdir,
    )
    y = np.stack([res.results[i]["y"] for i in range(NCORES)], axis=0)
    if _trace:
        _CACHE["last_result"] = res
    return y.reshape(B, T, D).astype(np.float32)


# revision 17
# speedup vs baseline: 4.6157x; 2.6244x over previous
# CrystalAttention Trainium2 kernel.
#
# Full inputs -> shard batch dim over 8 NeuronCores -> bass/Tile kernel ->
# gather. Per core: x_sh [2048, 512].
#
#   dist2[n,m] = |x[m]|^2 + |pos[n]|^2 - 2 x[m].pos[n]
#   attn = softmax_n(scales[n] / (sqrt(dist2) + 0.1))
#   y = (attn @ values) @ W_out^T + b_out
#
# Layout: scores kept transposed [n on partitions, m free] so the softmax
# numerator matrix e feeds mm2 (lhsT=values, rhs=e) with no runtime
# transposes of the big [M,N] tensor. Softmax has no max-subtraction
# (scores = scales/(dist+0.1) ~ 0.22, exp is tiny and safe), and the
# normalization by the denominator is deferred to after out_proj (it is a
# per-row scalar).
#
# Matmuls run in bf16 (PE full rate). The distance's large constant is kept
# out of bf16: the augmentation row carries x_sq-512 and the fp32 Sqrt bias
# carries p_sq+512, so quantization error stays ~0.1% of the small residual
# rather than of ~516.
import numpy as np

B, T, D, N = 8, 2048, 512, 1024
NCORES = 8
P = 128
M = (B * T) // NCORES      # 2048 rows per core
MT = 512                   # m tile (matmul moving free dim)
N_MT = M // MT             # 4
DC = D // P                # 4 contraction chunks of 128
NCH = N // P               # 8 neuron chunks of 128
MS = MT // P               # 4 m-subtiles per m tile
XSQ_C = 512.0              # E[|x|^2] offset kept in fp32 bias

_CACHE = {}


def _build_bass(reps=1):
    import concourse.bacc as bacc
    import concourse.tile as tile
    import concourse.mybir as mybir
    from concourse.masks import make_identity

    fp32 = mybir.dt.float32
    bf16 = mybir.dt.bfloat16
    fp8 = mybir.dt.float8e4
    DR = mybir.MatmulPerfMode.DoubleRow
    AF = mybir.ActivationFunctionType
    OP = mybir.AluOpType

    nc = bacc.Bacc(None, target_bir_lowering=False)

    x_d = nc.dram_tensor("x", [M, D], bf16, kind="ExternalInput")
    posn_d = nc.dram_tensor("posn", [D, N], fp8, kind="ExternalInput")    # -32*pos^T
    psq_d = nc.dram_tensor("psq", [P, NCH], fp32, kind="ExternalInput")   # |pos|^2+512
    invs_d = nc.dram_tensor("invs", [P, NCH], fp32, kind="ExternalInput")  # 1/scales
    offs_d = nc.dram_tensor("offs", [P, NCH], fp32, kind="ExternalInput")  # 0.1/scales
    val_d = nc.dram_tensor("vals", [N, D], fp8, kind="ExternalInput")     # 16*values
    wT_d = nc.dram_tensor("wT", [D, D], bf16, kind="ExternalInput")       # W_out^T
    bb_d = nc.dram_tensor("bb", [P, D], fp32, kind="ExternalInput")       # b_out bcast
    y_d = nc.dram_tensor("y", [M, D], fp32, kind="ExternalOutput")

    with tile.TileContext(nc) as tc:
        with (
            tc.tile_pool(name="const", bufs=1) as const,
            tc.tile_pool(name="big", bufs=1) as big,
            tc.tile_pool(name="sq", bufs=2) as sq_pool,
            tc.tile_pool(name="ut", bufs=2) as ut_pool,
            tc.tile_pool(name="small", bufs=3) as small,
            tc.tile_pool(name="yo", bufs=3) as y_pool,
            tc.tile_pool(name="ps_mm", bufs=6, space="PSUM") as ps_mm,
            tc.tile_pool(name="ps_row", bufs=1, space="PSUM") as ps_row,
            tc.tile_pool(name="ps_dt", bufs=1, space="PSUM") as ps_dt,
        ):
            # ---- constants / weights ----
            ident = const.tile([P, P], fp32)
            make_identity(nc, ident)
            ones_row = const.tile([1, P], bf16)    # aug stationary [k=1, n=128]
            nc.vector.memset(ones_row, 1.0)
            ones_col = const.tile([P, 1], bf16)    # reduce stationary [k=128, 1]
            nc.vector.memset(ones_col, 1.0)
            ones_col8 = const.tile([P, 1], fp8)
            nc.vector.memset(ones_col8, 1.0)

            posn_sb = const.tile([P, DC, N], fp8)
            nc.sync.dma_start(posn_sb, posn_d.rearrange("(c p) n -> p c n", p=P))
            psq_sb = const.tile([P, NCH], fp32)
            nc.sync.dma_start(psq_sb, psq_d[:])
            invs_sb = const.tile([P, NCH], fp32)
            nc.sync.dma_start(invs_sb, invs_d[:])
            offs_sb = const.tile([P, NCH], fp32)
            nc.sync.dma_start(offs_sb, offs_d[:])
            val_sb = const.tile([P, NCH, D], fp8)
            nc.sync.dma_start(val_sb, val_d.rearrange("(c p) d -> p c d", p=P))
            wT_sb = const.tile([P, DC, D], bf16)
            nc.sync.dma_start(wT_sb, wT_d.rearrange("(c p) o -> p c o", p=P))
            bb_sb = const.tile([P, D], fp32)
            nc.sync.dma_start(bb_sb, bb_d[:])

            warm = const.tile([1, 1], fp32)
            nc.scalar.activation(warm, ident[0:1, 0:1], AF.Sqrt)

            xT_sb = big.tile([P, DC, M], bf16)      # x transposed [d, m]
            xT8_sb = big.tile([P, DC, M], fp8)      # 16*x for fp8 mm1
            score_sb = big.tile([P, NCH, M], fp32)  # dist -> score
            e_sb = big.tile([P, NCH, M], fp8)       # exp(score)
            xsq_sb = big.tile([1, M], bf16)         # |x[m]|^2 - 512 row

            import contextlib
            loop_cm = (
                tc.For_i(0, reps, 1, hint_engines=(mybir.EngineType.PE,))
                if reps > 1 else contextlib.nullcontext()
            )
            sqrt_instrs = []

            # ================= phase A: distances + scores =================
            stack = contextlib.ExitStack()
            stack.enter_context(loop_cm)
            for t in range(N_MT):
                mt = slice(t * MT, (t + 1) * MT)
                # transpose x [m,d] -> xT [d,m] via XBAR DMA (bf16)
                nc.sync.dma_start_transpose(
                    xT_sb[:, :, mt], x_d[t * MT:(t + 1) * MT, :]
                )
                nc.scalar.activation(
                    xT8_sb[:, :, mt], xT_sb[:, :, mt], AF.Copy, scale=16.0
                )

                # |x|^2 row: square then ones-reduce over partitions
                sqt = sq_pool.tile([P, DC, MT], bf16, tag="sq")
                nc.scalar.activation(sqt, xT_sb[:, :, mt], AF.Square)
                xq = ps_row.tile([1, MT], fp32, tag="row")
                for dc in range(DC):
                    nc.tensor.matmul(
                        xq, ones_col, sqt[:, dc, :],
                        start=(dc == 0), stop=(dc == DC - 1),
                    )
                # xsq residual row in bf16: x_sq - 512
                nc.vector.tensor_scalar(
                    xsq_sb[:, mt], xq, -XSQ_C, 256.0, op0=OP.add, op1=OP.mult
                )

                # mm1 per neuron chunk: d2 = xsq + psq - 2 x.pos, then score
                for c in range(NCH):
                    d2 = ps_mm.tile([P, MT], fp32, tag="mm")
                    for dc in range(0, DC, 2):
                        nc.tensor.matmul(
                            d2,
                            posn_sb[:, dc:dc + 2, c * P:(c + 1) * P],
                            xT8_sb[:, dc:dc + 2, mt],
                            start=(dc == 0), stop=False,
                            perf_mode=DR,
                        )
                    nc.tensor.matmul(
                        d2, ones_row, xsq_sb[:, mt], start=False, stop=True
                    )
                    sc = score_sb[:, c, mt]
                    si = nc.scalar.activation(
                        sc, d2, AF.Sqrt, bias=psq_sb[:, c:c + 1], scale=1.0
                    )
                    sqrt_instrs.append(si)
                    # v = (dist + 0.1)/scales  (per-partition scalars)
                    nc.gpsimd.tensor_scalar(
                        sc, sc, invs_sb[:, c:c + 1], offs_sb[:, c:c + 1],
                        op0=OP.mult, op1=OP.add,
                    )
                    nc.vector.reciprocal_approx_fast(sc, sc)

            # ================= phase B: exp + attn@values + out_proj =======
            exp_instrs = []
            for t in range(N_MT):
                mt = slice(t * MT, (t + 1) * MT)
                ei = nc.scalar.activation(
                    e_sb[:, :, mt], score_sb[:, :, mt], AF.Exp
                )
                exp_instrs.append(ei)

                # uT[d, m] = sum_n values[n, d] e[n, m]
                ut = ut_pool.tile([P, DC, MT], bf16, tag="ut")
                for ds in range(DC):
                    u = ps_mm.tile([P, MT], fp32, tag="mm")
                    for c in range(0, NCH, 2):
                        nc.tensor.matmul(
                            u,
                            val_sb[:, c:c + 2, ds * P:(ds + 1) * P],
                            e_sb[:, c:c + 2, mt],
                            start=(c == 0), stop=(c == NCH - 2),
                            perf_mode=DR,
                        )
                    nc.vector.tensor_copy(ut[:, ds, :], u)

                # denom[m] = sum_n e[n, m]
                den = ps_row.tile([1, MT], fp32, tag="row")
                for c in range(NCH):
                    nc.tensor.matmul(
                        den, ones_col8, e_sb[:, c, mt],
                        start=(c == 0), stop=(c == NCH - 1),
                    )
                den_sb = small.tile([1, MT], fp32, tag="densb")
                nc.scalar.activation(den_sb, den, AF.Copy, scale=16.0)
                # transpose denom row into [m partitions, 1] and invert
                dt_ps = ps_dt.tile([P, MS], fp32, tag="dt")
                for j in range(MS):
                    nc.tensor.transpose(
                        dt_ps[:, j:j + 1], den_sb[:, j * P:(j + 1) * P],
                        ident[0:1, 0:1],
                    )
                rden = small.tile([P, MS], fp32, tag="rden")
                nc.vector.reciprocal_approx_fast(rden, dt_ps)

                # y = (uT^T @ W^T) * (1/denom) + b
                for ms in range(MS):
                    yps = ps_mm.tile([P, D], fp32, tag="mm")
                    for dc in range(DC):
                        nc.tensor.matmul(
                            yps,
                            ut[:, dc, ms * P:(ms + 1) * P],
                            wT_sb[:, dc, :],
                            start=(dc == 0), stop=(dc == DC - 1),
                        )
                    yt = y_pool.tile([P, D], fp32, tag="y")
                    nc.vector.scalar_tensor_tensor(
                        yt, yps, rden[:, ms:ms + 1], bb_sb,
                        op0=OP.mult, op1=OP.add,
                    )
                    r0 = t * MT + ms * P
                    nc.sync.dma_start(y_d[r0:r0 + P, :], yt)

            # keep every Exp after the last Sqrt on the ACT stream so the
            # activation table set switches exactly once
            last_sqrt = sqrt_instrs[-1]
            for ei in exp_instrs:
                tile.add_dep_helper(
                    ei.ins, last_sqrt.ins, sync=False, reason="act table phase"
                )
            stack.close()

    nc.compile()
    return nc


def _get_bass(reps=1):
    key = ("nc", reps)
    if key not in _CACHE:
        _CACHE[key] = _build_bass(reps)
    return _CACHE[key]


def _host_prep(positions, scales, values, W_out, b_out):
    import ml_dtypes

    bf16 = ml_dtypes.bfloat16
    fp8 = ml_dtypes.float8_e4m3
    positions = positions.astype(np.float32)
    # fp8 operands carry a 16x scale on each side; distances come out 256x
    posn = np.ascontiguousarray((-32.0 * positions.T).astype(fp8))
    psq = np.ascontiguousarray(
        (256.0 * ((positions ** 2).sum(-1) + XSQ_C))
        .astype(np.float32).reshape(NCH, P).T
    )
    invs = np.ascontiguousarray(
        (1.0 / (16.0 * scales)).astype(np.float32).reshape(NCH, P).T
    )
    offs = np.ascontiguousarray((0.1 / scales).astype(np.float32).reshape(NCH, P).T)
    vals = np.ascontiguousarray((16.0 * values.astype(np.float32)).astype(fp8))
    wT = np.ascontiguousarray(W_out.astype(np.float32).T.astype(bf16))
    bb = np.ascontiguousarray(np.tile(b_out.astype(np.float32)[None, :], (P, 1)))
    return posn, psq, invs, offs, vals, wT, bb


def kernel(x, positions, scales, values, W_out, b_out, _trace=False, _tmpdir=None,
           _reps=1):
    from concourse.bass_utils import run_bass_kernel_spmd

    import ml_dtypes
    x = np.asarray(x, dtype=np.float32).reshape(B * T, D).astype(ml_dtypes.bfloat16)
    posn, psq, invs, offs, vals, wT, bb = _host_prep(
        np.asarray(positions), np.asarray(scales), np.asarray(values),
        np.asarray(W_out), np.asarray(b_out),
    )

    shards = x.reshape(NCORES, M, D)
    in_maps = [
        {
            "x": np.ascontiguousarray(shards[i]),
            "posn": posn, "psq": psq, "invs": invs, "offs": offs,
            "vals": vals, "wT": wT, "bb": bb,
        }
        for i in range(NCORES)
    ]

    nc = _get_bass(_reps)
    res = run_bass_kernel_spmd(
        nc, in_maps, core_ids=list(range(NCORES)),
        trace=_trace, tmpdir=_tmpdir,
    )
    y = np.stack([res.results[i]["y"] for i in range(NCORES)], axis=0)
    if _trace:
        _CACHE["last_result"] = res
    return y.reshape(B, T, D).astype(np.float32)
